# revision 11
# baseline (speedup 1.0000x reference)
"""Trainium2 Bass kernel for CoExDispProcessor (topk_masking) — v2.

Per-sample (data-parallel over batch across 8 cores):
  1. top-2 over D=48 of cost [48,128,240] -> softmax blend -> disp4 [128,240]
  2. 3x3 unfold of disp4 -> nearest 4x upsample -> weighted sum with
     softmax over 9 channels of spg [9,512,960] -> disp1 [512,960]

v2 architecture:
  - top-2: DVE max8 + max_index per w column (exact fp32 ordering), batched
    full-width blend (~2us) -> disp4p [128, 242] fp16 (zero-padded cols).
  - fine stage: partition = (cj, ci, y) channels-on-partition row tiles
    (36x14 rows + 1x8). TensorE builds the upsampled 3x3 patch tensor u via
    ones-matmuls against disp4p windows (zero-padded lhsT accumulated over
    cj), and reduces the 9 channels of num/den via ones-matmuls into PSUM
    group accumulators. ScalarE: exps + psum->sbuf expand/copies. Muls
    e*u split DVE/GpSimd. Final 4*num/den on DVE, out DMA per 126-row group.
  - 0/1 weight matrices (5 wu patterns, wsum, wsum36) are passed as extra
    const inputs and DMA'd once (~8KB).
"""

import os
import sys
from contextlib import ExitStack

import numpy as np

if "/opt/trn_rl_repo" not in sys.path:
    sys.path.insert(0, "/opt/trn_rl_repo")

import concourse.bass as bass
import concourse.bacc as bacc
import concourse.tile as tile
from concourse import mybir
from concourse.bass_utils import run_bass_kernel_spmd

F32 = mybir.dt.float32
FP16 = mybir.dt.float16
U16 = mybir.dt.uint16
OP = mybir.AluOpType
ACT = mybir.ActivationFunctionType

B, D, H, W = 8, 48, 128, 240
HF, WF = 4 * H, 4 * W  # 512, 960
N_CORES = 8
NT = 37  # 36 tiles of 14 fine rows + 1 tile of 8

COST_CHUNKS = [32, 96, 112]  # w-column DMA chunks (first small for early start)

# knobs
def is_coarse(t):  # coarse-u (ScalarE expand) vs fine-u (TensorE stride-0)
    return t % 2 == 0

def mul_on_gp(t):  # which engine runs p = e*u
    return t % 3 != 1


def nrows(t):
    return 14 if t < 36 else 8


def build_consts():
    """wu [128,37,210] zero-padded sliding patterns, wsum, wsum36 (fp16 0/1).

    lhsT for (t, cj) = wu[:, t, 84-3*nb*cj : 84-3*nb*cj+M] — the base (ci,yl)
    pattern sits at cols 84..84+3*nb, so the window zero-masks all but the
    cj-th slab of the (cj,ci,yl) output layout."""
    wu = np.zeros((128, NT, 210), dtype=np.float16)
    for t in range(NT):
        nr = nrows(t)
        nb = nr
        for ci in range(3):
            for yl in range(nr):
                r = (14 * t + yl) // 4 + ci - 1
                if 0 <= r <= 127:
                    wu[r, t, 84 + nb * ci + yl] = 1.0
    wsum = np.zeros((126, 9, 126), dtype=np.float16)
    for cj in range(3):
        for ci in range(3):
            for yl in range(14):
                p = 42 * cj + 14 * ci + yl
                for j in range(9):
                    wsum[p, j, 14 * j + yl] = 1.0
    wsum36 = np.zeros((72, 8), dtype=np.float16)
    for cj in range(3):
        for ci in range(3):
            for yl in range(8):
                wsum36[24 * cj + 8 * ci + yl, yl] = 1.0
    return wu, wsum, wsum36


def build_kernel(ctx, tc, out_d, cost_d, spg_d, wu_d, wsum_d, wsum36_d,
                 dbg=None):
    nc = tc.nc
    cost_hdw = cost_d.transpose([1, 0, 2])  # [128(h), 48(d), 240(w)] view
    spg_v = spg_d.rearrange("(ci cj) Y x -> cj ci Y x", ci=3, cj=3)

    persist = ctx.enter_context(tc.tile_pool(name="persist", bufs=1))
    raw_pool = ctx.enter_context(tc.tile_pool(name="raw", bufs=3))
    e_pool = ctx.enter_context(tc.tile_pool(name="epool", bufs=NT))
    den_pool = ctx.enter_context(tc.tile_pool(name="densb", bufs=5))
    u_pool = ctx.enter_context(tc.tile_pool(name="usb", bufs=3))
    p_pool = ctx.enter_context(tc.tile_pool(name="ppool", bufs=3))
    r_pool = ctx.enter_context(tc.tile_pool(name="rpool", bufs=2))
    o_pool = ctx.enter_context(tc.tile_pool(name="opool", bufs=2))
    acc_ps = ctx.enter_context(tc.tile_pool(name="accps", bufs=2, space="PSUM"))
    u_ps_pool = ctx.enter_context(tc.tile_pool(name="ups", bufs=2, space="PSUM"))

    # ---- persistent tiles ----
    ctile = persist.tile([128, D, W], F32)
    v8 = persist.tile([128, W, 8], F32)
    i8 = persist.tile([128, W, 8], U16)
    disp4p = persist.tile([128, W + 2], FP16)
    bl = [persist.tile([128, W], F32, name=f"bl{i}", tag=f"bl{i}")
          for i in range(7)]
    wu_sb = persist.tile([128, NT, 210], FP16)
    wsum_sb = persist.tile([126, 9, 126], FP16)
    wsum36_sb = persist.tile([72, 8], FP16)

    # ---- const DMAs + border memset ----
    nc.sync.dma_start(wu_sb[:], wu_d)
    nc.sync.dma_start(wsum_sb[:], wsum_d)
    nc.sync.dma_start(wsum36_sb[:], wsum36_d)
    nc.vector.memset(disp4p[:], 0.0)

    # ---- cost DMAs (scalar HWDGE ring, first chunk small) ----
    w0 = 0
    chunk_bounds = []
    for nw in COST_CHUNKS:
        nc.scalar.dma_start(ctile[:, :, w0:w0 + nw], cost_hdw[:, :, w0:w0 + nw])
        chunk_bounds.append((w0, w0 + nw))
        w0 += nw

    # ---- spg raw DMAs (both rings, t order) ----
    raw_tiles = {}
    for t in range(NT):
        nr = nrows(t)
        raw = raw_pool.tile([9 * nr, WF], F32, tag="raw")
        eng = nc.sync if t % 2 == 0 else nc.scalar
        for cj in range(3):
            eng.dma_start(
                raw[3 * nr * cj:3 * nr * (cj + 1), :],
                spg_v[cj, :, 14 * t:14 * t + nr, :],
            )
        raw_tiles[t] = raw

    # ---- exps (SE) + den matmuls (TE) + den copies (SE) ----
    e_tiles = {}
    den_sb = {}
    den_ps = {}
    for t in range(NT):
        nr = nrows(t)
        P = 9 * nr
        e_t = e_pool.tile([P, WF], FP16, name=f"e{t}", tag="e")
        nc.scalar.activation(e_t[:], raw_tiles[t][:], ACT.Exp)
        e_tiles[t] = e_t
        g, j = (t // 9, t % 9) if t < 36 else (4, 0)
        if j == 0:
            den_ps[g] = acc_ps.tile([(8 if t == 36 else 126), WF], F32,
                                    name=f"denps{g}", tag="acc")
        lhsT = wsum36_sb[:] if t == 36 else wsum_sb[:, j, :]
        last = (j == 8) or (t == 36)
        for (n0, nn) in ((0, 512), (512, 448)):
            nc.tensor.matmul(
                den_ps[g][:, n0:n0 + nn], lhsT, e_t[:, n0:n0 + nn],
                start=(j == 0), stop=last,
            )
        if last:
            dsb = den_pool.tile([den_ps[g].shape[0], WF], FP16, name=f"dsb{g}", tag="dsb")
            nc.scalar.copy(dsb[:], den_ps[g][:])
            den_sb[g] = dsb
            if dbg is not None and g == 0:
                nc.sync.dma_start(dbg["den0"], dsb[:])

    # ---- top-2 stream (DVE) ----
    for (a, b) in chunk_bounds:
        for w in range(a, b):
            nc.vector.max(out=v8[:, w], in_=ctile[:, :, w])
        for w in range(a, b):
            nc.vector.max_index(i8[:, w], v8[:, w], ctile[:, :, w])

    # ---- blend (DVE + one SE exp) -> disp4p[:, 1:241] ----
    i1f, i2f, dl, tx, dn, rc, nm = bl
    nc.vector.tensor_copy(i1f[:], i8[:, :, 0])
    nc.vector.tensor_copy(i2f[:], i8[:, :, 1])
    nc.vector.tensor_sub(dl[:], v8[:, :, 1], v8[:, :, 0])
    nc.scalar.activation(tx[:], dl[:], ACT.Exp)
    nc.vector.tensor_scalar_add(dn[:], tx[:], 1.0)
    nc.vector.reciprocal(rc[:], dn[:])
    nc.vector.tensor_mul(nm[:], tx[:], i2f[:])
    nc.vector.tensor_add(nm[:], nm[:], i1f[:])
    nc.vector.tensor_mul(disp4p[:, 1:241], nm[:], rc[:])

    # ---- fine-stage tail ----
    num_ps = {}

    def emit_u_and_mul(t):
        _dbg = dbg
        nr = nrows(t)
        P = 9 * nr
        M = 126 if t < 36 else 72
        if is_coarse(t):
            ups_full = u_ps_pool.tile([M, WF], F32, name=f"ups{t}", tag="u")
            ups = ups_full[:, :W]
            for cj in range(3):
                s = 84 - 3 * nr * cj
                nc.tensor.matmul(
                    ups, wu_sb[:, t, s:s + M], disp4p[:, cj:cj + W],
                    start=(cj == 0), stop=(cj == 2),
                )
            usb = u_pool.tile([M, WF], FP16, tag="u")
            nc.scalar.copy(
                usb[:].rearrange("q (a b) -> q a b", b=4),
                ups.unsqueeze(2).broadcast_to([M, W, 4]),
            )
        else:
            ups = u_ps_pool.tile([M, WF], F32, name=f"ups{t}", tag="u")
            for cj in range(3):
                s = 84 - 3 * nr * cj
                for (n0, nn) in ((0, 512), (512, 448)):
                    rhs = disp4p[:, cj + n0 // 4: cj + (n0 + nn) // 4]
                    rhs = rhs.unsqueeze(2).broadcast_to([128, nn // 4, 4])
                    nc.tensor.matmul(
                        ups[:, n0:n0 + nn], wu_sb[:, t, s:s + M], rhs,
                        start=(cj == 0), stop=(cj == 2),
                    )
            usb = u_pool.tile([M, WF], FP16, tag="u")
            nc.scalar.copy(usb[:], ups[:])
        p_t = p_pool.tile([P, WF], FP16, tag="p")
        eng = nc.gpsimd if mul_on_gp(t) else nc.vector
        eng.tensor_mul(p_t[:], e_tiles[t][:], usb[:M])
        if _dbg is not None and t == 0:
            nc.sync.dma_start(_dbg["u0"], usb[:])
            nc.sync.dma_start(_dbg["e0"], e_tiles[t][:])
            nc.sync.dma_start(_dbg["p0"], p_t[:])
        return p_t

    def emit_num(t, p_t):
        g, j = (t // 9, t % 9) if t < 36 else (4, 0)
        if j == 0:
            num_ps[g] = acc_ps.tile([(8 if t == 36 else 126), WF], F32,
                                    name=f"numps{g}", tag="acc")
        lhsT = wsum36_sb[:] if t == 36 else wsum_sb[:, j, :]
        last = (j == 8) or (t == 36)
        for (n0, nn) in ((0, 512), (512, 448)):
            nc.tensor.matmul(
                num_ps[g][:, n0:n0 + nn], lhsT, p_t[:, n0:n0 + nn],
                start=(j == 0), stop=last,
            )

    def emit_div_out(g):
        rows = 126 if g < 4 else 8
        rden = r_pool.tile([rows, WF], F32, tag="rd")
        nc.vector.reciprocal(rden[:], den_sb[g][:])
        osb = o_pool.tile([rows, WF], F32, tag="o")
        nc.vector.scalar_tensor_tensor(
            osb[:], num_ps[g][:], 4.0, rden[:], op0=OP.mult, op1=OP.mult,
        )
        nc.sync.dma_start(out_d[126 * g:126 * g + rows, :], osb[:])

    if dbg is not None:
        nc.sync.dma_start(dbg["disp4p"], disp4p[:])
    prev_p = None
    for t in range(NT):
        p_t = emit_u_and_mul(t)
        if prev_p is not None:
            emit_num(t - 1, prev_p)
            if t % 9 == 0 and t > 0:
                emit_div_out(t // 9 - 1)
        prev_p = p_t
    emit_num(NT - 1, prev_p)
    emit_div_out(3)
    emit_div_out(4)


def build_program():
    nc = bacc.Bacc(
        "TRN2",
        target_bir_lowering=False,
        debug=False,
        enable_asserts=False,
        num_devices=N_CORES,
    )
    cost_d = nc.dram_tensor("cost", [D, H, W], F32, kind="ExternalInput").ap()
    spg_d = nc.dram_tensor("spg", [9, HF, WF], F32, kind="ExternalInput").ap()
    wu_d = nc.dram_tensor("wu", [128, NT, 210], FP16, kind="ExternalInput").ap()
    wsum_d = nc.dram_tensor("wsum", [126, 9, 126], FP16, kind="ExternalInput").ap()
    wsum36_d = nc.dram_tensor("wsum36", [72, 8], FP16, kind="ExternalInput").ap()
    out_d = nc.dram_tensor("out", [HF, WF], F32, kind="ExternalOutput").ap()
    dbg = None
    if os.environ.get("KERNEL_DEBUG"):
        dbg = {
            "disp4p": nc.dram_tensor("dbg_disp4p", [128, 242], FP16, kind="ExternalOutput").ap(),
            "u0": nc.dram_tensor("dbg_u0", [126, WF], FP16, kind="ExternalOutput").ap(),
            "e0": nc.dram_tensor("dbg_e0", [126, WF], FP16, kind="ExternalOutput").ap(),
            "p0": nc.dram_tensor("dbg_p0", [126, WF], FP16, kind="ExternalOutput").ap(),
            "den0": nc.dram_tensor("dbg_den0", [126, WF], FP16, kind="ExternalOutput").ap(),
        }
    with tile.TileContext(nc) as tc:
        with ExitStack() as ctx:
            build_kernel(ctx, tc, out_d, cost_d, spg_d, wu_d, wsum_d, wsum36_d,
                         dbg=dbg)
    nc.compile()
    return nc


def _install_ntff_hook():
    import types

    if "antenv.axon_hooks" in sys.modules:
        return True
    try:
        import antenv
        from trn_agent_boot.trn_boot import _ntff_profile_via_ctypes

        mod = types.ModuleType("antenv.axon_hooks")
        mod._hook = None

        def set_axon_ntff_profile_hook(hook):
            mod._hook = hook

        def get_axon_ntff_profile_hook():
            return mod._hook

        mod.set_axon_ntff_profile_hook = set_axon_ntff_profile_hook
        mod.get_axon_ntff_profile_hook = get_axon_ntff_profile_hook
        sys.modules["antenv.axon_hooks"] = mod
        antenv.axon_hooks = mod
        mod._hook = _ntff_profile_via_ctypes("/opt/axon/libaxon_pjrt.so")
        return True
    except Exception as e:  # profiling is best-effort
        print(f"NTFF hook install failed: {e}")
        return False


LAST_RESULTS = None


def kernel(cost: np.ndarray, spg: np.ndarray) -> np.ndarray:
    """cost [8,1,48,128,240] f32, spg [8,9,512,960] f32 -> disp1 [8,512,960]."""
    global LAST_RESULTS
    cost = np.ascontiguousarray(np.asarray(cost, dtype=np.float32))
    spg = np.ascontiguousarray(np.asarray(spg, dtype=np.float32))
    assert cost.shape == (B, 1, D, H, W) and spg.shape == (B, 9, HF, WF)

    wu, wsum, wsum36 = build_consts()
    nc = build_program()
    in_maps = [
        {"cost": cost[b, 0], "spg": spg[b],
         "wu": wu, "wsum": wsum, "wsum36": wsum36}
        for b in range(B)
    ]
    trace = bool(int(os.environ.get("KERNEL_TRACE", "0")))
    if trace:
        trace = _install_ntff_hook()
    res = run_bass_kernel_spmd(
        nc, in_maps, core_ids=list(range(N_CORES)), trace=trace
    )
    LAST_RESULTS = res
    out = np.stack([res.results[b]["out"] for b in range(B)], axis=0)
    return out.astype(np.float32, copy=False)


# revision 12
# speedup vs baseline: 1.0288x; 1.0288x over previous
"""Trainium2 Bass kernel for CoExDispProcessor (topk_masking) — v2.

Per-sample (data-parallel over batch across 8 cores):
  1. top-2 over D=48 of cost [48,128,240] -> softmax blend -> disp4 [128,240]
  2. 3x3 unfold of disp4 -> nearest 4x upsample -> weighted sum with
     softmax over 9 channels of spg [9,512,960] -> disp1 [512,960]

v2 architecture:
  - top-2: DVE max8 + max_index per w column (exact fp32 ordering), batched
    full-width blend (~2us) -> disp4p [128, 242] fp16 (zero-padded cols).
  - fine stage: partition = (cj, ci, y) channels-on-partition row tiles
    (36x14 rows + 1x8). TensorE builds the upsampled 3x3 patch tensor u via
    ones-matmuls against disp4p windows (zero-padded lhsT accumulated over
    cj), and reduces the 9 channels of num/den via ones-matmuls into PSUM
    group accumulators. ScalarE: exps + psum->sbuf expand/copies. Muls
    e*u split DVE/GpSimd. Final 4*num/den on DVE, out DMA per 126-row group.
  - 0/1 weight matrices (5 wu patterns, wsum, wsum36) are passed as extra
    const inputs and DMA'd once (~8KB).
"""

import os
import sys
from contextlib import ExitStack

import numpy as np
import ml_dtypes

if "/opt/trn_rl_repo" not in sys.path:
    sys.path.insert(0, "/opt/trn_rl_repo")

import concourse.bass as bass
import concourse.bacc as bacc
import concourse.tile as tile
from concourse import mybir
from concourse.bass_utils import run_bass_kernel_spmd

F32 = mybir.dt.float32
FP16 = mybir.dt.float16
BF16 = mybir.dt.bfloat16
U16 = mybir.dt.uint16
OP = mybir.AluOpType
ACT = mybir.ActivationFunctionType

B, D, H, W = 8, 48, 128, 240
HF, WF = 4 * H, 4 * W  # 512, 960
N_CORES = 8
NT = 37  # 36 tiles of 14 fine rows + 1 tile of 8

COST_CHUNKS = [32, 96, 112]  # w-column DMA chunks (first small for early start)

# knobs
def is_coarse(t):  # coarse-u (ScalarE expand) vs fine-u (TensorE stride-0)
    return t % 2 == 0

def mul_on_gp(t):  # which engine runs p = e*u
    return t % 2 == 1


def nrows(t):
    return 14 if t < 36 else 8


def build_consts():
    """wu [128,37,210] zero-padded sliding patterns, wsum, wsum36 (fp16 0/1).

    lhsT for (t, cj) = wu[:, t, 84-3*nb*cj : 84-3*nb*cj+M] — the base (ci,yl)
    pattern sits at cols 84..84+3*nb, so the window zero-masks all but the
    cj-th slab of the (cj,ci,yl) output layout."""
    wu = np.zeros((128, NT, 210), dtype=ml_dtypes.bfloat16)
    for t in range(NT):
        nr = nrows(t)
        nb = nr
        for ci in range(3):
            for yl in range(nr):
                r = (14 * t + yl) // 4 + ci - 1
                if 0 <= r <= 127:
                    wu[r, t, 84 + nb * ci + yl] = 1.0
    wsum = np.zeros((126, 9, 126), dtype=ml_dtypes.bfloat16)
    for cj in range(3):
        for ci in range(3):
            for yl in range(14):
                p = 42 * cj + 14 * ci + yl
                for j in range(9):
                    wsum[p, j, 14 * j + yl] = 1.0
    wsum36 = np.zeros((72, 8), dtype=ml_dtypes.bfloat16)
    for cj in range(3):
        for ci in range(3):
            for yl in range(8):
                wsum36[24 * cj + 8 * ci + yl, yl] = 1.0
    return wu, wsum, wsum36


def _act_reciprocal(nc, out_ap, in_ap):
    eng = nc.scalar
    return eng.add_instruction(
        mybir.InstActivation(
            name=nc.get_next_instruction_name(),
            func=ACT.Reciprocal,
            ins=[
                eng.lower_ap(in_ap),
                mybir.ImmediateValue(dtype=F32, value=0.0),
                mybir.ImmediateValue(dtype=F32, value=1.0),
                mybir.ImmediateValue(dtype=F32, value=0.0),
            ],
            outs=[eng.lower_ap(out_ap)],
        )
    )


def build_kernel(ctx, tc, out_d, cost_d, spg_d, wu_d, wsum_d, wsum36_d,
                 dbg=None):
    nc = tc.nc
    cost_hdw = cost_d.transpose([1, 0, 2])  # [128(h), 48(d), 240(w)] view
    spg_v = spg_d.rearrange("(ci cj) Y x -> cj ci Y x", ci=3, cj=3)

    persist = ctx.enter_context(tc.tile_pool(name="persist", bufs=1))
    raw_pool = ctx.enter_context(tc.tile_pool(name="raw", bufs=3))
    e_pool = ctx.enter_context(tc.tile_pool(name="epool", bufs=NT))
    den_pool = ctx.enter_context(tc.tile_pool(name="densb", bufs=5))
    u_pool = ctx.enter_context(tc.tile_pool(name="usb", bufs=3))
    p_pool = ctx.enter_context(tc.tile_pool(name="ppool", bufs=3))
    r_pool = ctx.enter_context(tc.tile_pool(name="rpool", bufs=2))
    o_pool = ctx.enter_context(tc.tile_pool(name="opool", bufs=2))
    acc_ps = ctx.enter_context(tc.tile_pool(name="accps", bufs=2, space="PSUM"))
    u_ps_pool = ctx.enter_context(tc.tile_pool(name="ups", bufs=2, space="PSUM"))

    # ---- persistent tiles ----
    ctile = persist.tile([128, D, W], F32)
    v8 = persist.tile([128, W, 8], F32)
    i8 = persist.tile([128, W, 8], U16)
    disp4p = persist.tile([128, W + 2], BF16)
    bl = [persist.tile([128, W], F32, name=f"bl{i}", tag=f"bl{i}")
          for i in range(7)]
    wu_sb = persist.tile([128, NT, 210], BF16)
    wsum_sb = persist.tile([126, 9, 126], BF16)
    wsum36_sb = persist.tile([72, 8], BF16)

    # ---- const DMAs + border memset ----
    nc.sync.dma_start(wu_sb[:], wu_d)
    nc.sync.dma_start(wsum_sb[:], wsum_d)
    nc.sync.dma_start(wsum36_sb[:], wsum36_d)
    nc.vector.memset(disp4p[:], 0.0)

    # ---- cost DMAs (scalar HWDGE ring, first chunk small) ----
    w0 = 0
    chunk_bounds = []
    for nw in COST_CHUNKS:
        nc.sync.dma_start(ctile[:, :, w0:w0 + nw], cost_hdw[:, :, w0:w0 + nw])
        chunk_bounds.append((w0, w0 + nw))
        w0 += nw

    # ---- spg raw DMAs (both rings, t order) ----
    raw_tiles = {}
    for t in range(NT):
        nr = nrows(t)
        raw = raw_pool.tile([9 * nr, WF], F32, tag="raw")
        for cj in range(3):
            nc.sync.dma_start(
                raw[3 * nr * cj:3 * nr * (cj + 1), :],
                spg_v[cj, :, 14 * t:14 * t + nr, :],
            )
        raw_tiles[t] = raw

    # ---- exps (SE) + den matmuls (TE) + den copies (SE) ----
    e_tiles = {}
    den_sb = {}
    den_ps = {}
    for t in range(NT):
        nr = nrows(t)
        P = 9 * nr
        e_t = e_pool.tile([P, WF], BF16, name=f"e{t}", tag="e")
        nc.scalar.activation(e_t[:], raw_tiles[t][:], ACT.Exp)
        e_tiles[t] = e_t
        g, j = (t // 9, t % 9) if t < 36 else (4, 0)
        if j == 0:
            den_ps[g] = acc_ps.tile([(8 if t == 36 else 126), WF], F32,
                                    name=f"denps{g}", tag="acc")
        lhsT = wsum36_sb[:] if t == 36 else wsum_sb[:, j, :]
        last = (j == 8) or (t == 36)
        for (n0, nn) in ((0, 512), (512, 448)):
            nc.tensor.matmul(
                den_ps[g][:, n0:n0 + nn], lhsT, e_t[:, n0:n0 + nn],
                start=(j == 0), stop=last,
            )
        if last:
            dsb = den_pool.tile([den_ps[g].shape[0], WF], BF16, name=f"dsb{g}", tag="dsb")
            nc.scalar.copy(dsb[:], den_ps[g][:])
            den_sb[g] = dsb
            if dbg is not None and g == 0:
                nc.sync.dma_start(dbg["den0"], dsb[:])

    # ---- top-2 stream (DVE) ----
    for (a, b) in chunk_bounds:
        for w in range(a, b):
            nc.vector.max(out=v8[:, w], in_=ctile[:, :, w])
        for w in range(a, b):
            nc.vector.max_index(i8[:, w], v8[:, w], ctile[:, :, w])

    # ---- blend (DVE + one SE exp) -> disp4p[:, 1:241] ----
    i1f, i2f, dl, tx, dn, rc, nm = bl
    nc.vector.tensor_copy(i1f[:], i8[:, :, 0])
    nc.vector.tensor_copy(i2f[:], i8[:, :, 1])
    nc.vector.tensor_sub(dl[:], v8[:, :, 1], v8[:, :, 0])
    nc.scalar.activation(tx[:], dl[:], ACT.Exp)
    nc.vector.tensor_scalar_add(dn[:], tx[:], 1.0)
    nc.vector.reciprocal(rc[:], dn[:])
    nc.vector.tensor_mul(nm[:], tx[:], i2f[:])
    nc.vector.tensor_add(nm[:], nm[:], i1f[:])
    nc.vector.tensor_mul(disp4p[:, 1:241], nm[:], rc[:])

    # ---- fine-stage tail ----
    num_ps = {}

    def emit_u_and_mul(t):
        _dbg = dbg
        nr = nrows(t)
        P = 9 * nr
        M = 126 if t < 36 else 72
        if is_coarse(t):
            ups_full = u_ps_pool.tile([M, WF], F32, name=f"ups{t}", tag="u")
            ups = ups_full[:, :W]
            for cj in range(3):
                s = 84 - 3 * nr * cj
                nc.tensor.matmul(
                    ups, wu_sb[:, t, s:s + M], disp4p[:, cj:cj + W],
                    start=(cj == 0), stop=(cj == 2),
                )
            usb = u_pool.tile([M, WF], BF16, tag="u")
            nc.scalar.copy(
                usb[:].rearrange("q (a b) -> q a b", b=4),
                ups.unsqueeze(2).broadcast_to([M, W, 4]),
            )
        else:
            ups = u_ps_pool.tile([M, WF], F32, name=f"ups{t}", tag="u")
            for cj in range(3):
                s = 84 - 3 * nr * cj
                for (n0, nn) in ((0, 512), (512, 448)):
                    rhs = disp4p[:, cj + n0 // 4: cj + (n0 + nn) // 4]
                    rhs = rhs.unsqueeze(2).broadcast_to([128, nn // 4, 4])
                    nc.tensor.matmul(
                        ups[:, n0:n0 + nn], wu_sb[:, t, s:s + M], rhs,
                        start=(cj == 0), stop=(cj == 2),
                    )
            usb = u_pool.tile([M, WF], BF16, tag="u")
            nc.scalar.copy(usb[:], ups[:])
        p_t = p_pool.tile([P, WF], BF16, tag="p")
        eng = nc.gpsimd if mul_on_gp(t) else nc.vector
        eng.tensor_mul(p_t[:], e_tiles[t][:], usb[:M])
        if _dbg is not None and t == 0:
            nc.sync.dma_start(_dbg["u0"], usb[:])
            nc.sync.dma_start(_dbg["e0"], e_tiles[t][:])
            nc.sync.dma_start(_dbg["p0"], p_t[:])
        return p_t

    def emit_num(t, p_t):
        g, j = (t // 9, t % 9) if t < 36 else (4, 0)
        if j == 0:
            num_ps[g] = acc_ps.tile([(8 if t == 36 else 126), WF], F32,
                                    name=f"numps{g}", tag="acc")
        lhsT = wsum36_sb[:] if t == 36 else wsum_sb[:, j, :]
        last = (j == 8) or (t == 36)
        for (n0, nn) in ((0, 512), (512, 448)):
            nc.tensor.matmul(
                num_ps[g][:, n0:n0 + nn], lhsT, p_t[:, n0:n0 + nn],
                start=(j == 0), stop=last,
            )

    def emit_div_out(g):
        rows = 126 if g < 4 else 8
        rden = r_pool.tile([rows, WF], F32, tag="rd")
        _act_reciprocal(nc, rden[:], den_sb[g][:])
        osb = o_pool.tile([rows, WF], F32, tag="o")
        nc.vector.scalar_tensor_tensor(
            osb[:], num_ps[g][:], 4.0, rden[:], op0=OP.mult, op1=OP.mult,
        )
        nc.sync.dma_start(out_d[126 * g:126 * g + rows, :], osb[:])

    if dbg is not None:
        nc.sync.dma_start(dbg["disp4p"], disp4p[:])
    prev_p = None
    for t in range(NT):
        p_t = emit_u_and_mul(t)
        if prev_p is not None:
            emit_num(t - 1, prev_p)
            if t % 9 == 0 and t > 0:
                emit_div_out(t // 9 - 1)
        prev_p = p_t
    emit_num(NT - 1, prev_p)
    emit_div_out(3)
    emit_div_out(4)


def build_program():
    nc = bacc.Bacc(
        "TRN2",
        target_bir_lowering=False,
        debug=False,
        enable_asserts=False,
        num_devices=N_CORES,
    )
    cost_d = nc.dram_tensor("cost", [D, H, W], F32, kind="ExternalInput").ap()
    spg_d = nc.dram_tensor("spg", [9, HF, WF], F32, kind="ExternalInput").ap()
    wu_d = nc.dram_tensor("wu", [128, NT, 210], BF16, kind="ExternalInput").ap()
    wsum_d = nc.dram_tensor("wsum", [126, 9, 126], BF16, kind="ExternalInput").ap()
    wsum36_d = nc.dram_tensor("wsum36", [72, 8], BF16, kind="ExternalInput").ap()
    out_d = nc.dram_tensor("out", [HF, WF], F32, kind="ExternalOutput").ap()
    dbg = None
    if os.environ.get("KERNEL_DEBUG"):
        dbg = {
            "disp4p": nc.dram_tensor("dbg_disp4p", [128, 242], BF16, kind="ExternalOutput").ap(),
            "u0": nc.dram_tensor("dbg_u0", [126, WF], BF16, kind="ExternalOutput").ap(),
            "e0": nc.dram_tensor("dbg_e0", [126, WF], BF16, kind="ExternalOutput").ap(),
            "p0": nc.dram_tensor("dbg_p0", [126, WF], BF16, kind="ExternalOutput").ap(),
            "den0": nc.dram_tensor("dbg_den0", [126, WF], BF16, kind="ExternalOutput").ap(),
        }
    with tile.TileContext(nc) as tc:
        with ExitStack() as ctx:
            build_kernel(ctx, tc, out_d, cost_d, spg_d, wu_d, wsum_d, wsum36_d,
                         dbg=dbg)
    nc.compile()
    return nc


def _install_ntff_hook():
    import types

    if "antenv.axon_hooks" in sys.modules:
        return True
    try:
        import antenv
        from trn_agent_boot.trn_boot import _ntff_profile_via_ctypes

        mod = types.ModuleType("antenv.axon_hooks")
        mod._hook = None

        def set_axon_ntff_profile_hook(hook):
            mod._hook = hook

        def get_axon_ntff_profile_hook():
            return mod._hook

        mod.set_axon_ntff_profile_hook = set_axon_ntff_profile_hook
        mod.get_axon_ntff_profile_hook = get_axon_ntff_profile_hook
        sys.modules["antenv.axon_hooks"] = mod
        antenv.axon_hooks = mod
        mod._hook = _ntff_profile_via_ctypes("/opt/axon/libaxon_pjrt.so")
        return True
    except Exception as e:  # profiling is best-effort
        print(f"NTFF hook install failed: {e}")
        return False


LAST_RESULTS = None


def kernel(cost: np.ndarray, spg: np.ndarray) -> np.ndarray:
    """cost [8,1,48,128,240] f32, spg [8,9,512,960] f32 -> disp1 [8,512,960]."""
    global LAST_RESULTS
    cost = np.ascontiguousarray(np.asarray(cost, dtype=np.float32))
    spg = np.ascontiguousarray(np.asarray(spg, dtype=np.float32))
    assert cost.shape == (B, 1, D, H, W) and spg.shape == (B, 9, HF, WF)

    wu, wsum, wsum36 = build_consts()
    nc = build_program()
    in_maps = [
        {"cost": cost[b, 0], "spg": spg[b],
         "wu": wu, "wsum": wsum, "wsum36": wsum36}
        for b in range(B)
    ]
    trace = bool(int(os.environ.get("KERNEL_TRACE", "0")))
    if trace:
        trace = _install_ntff_hook()
    res = run_bass_kernel_spmd(
        nc, in_maps, core_ids=list(range(N_CORES)), trace=trace
    )
    LAST_RESULTS = res
    out = np.stack([res.results[b]["out"] for b in range(B)], axis=0)
    return out.astype(np.float32, copy=False)


# revision 17
# speedup vs baseline: 1.9769x; 1.9216x over previous
"""Trainium2 Bass kernel for CoExDispProcessor (topk_masking) — v3.

Per-sample (data-parallel over batch across 8 cores):
  1. top-2 over D=48 of cost [48,128,240] -> softmax blend -> disp4 [128,240]
  2. 3x3 unfold of disp4 -> nearest 4x upsample -> weighted sum with
     softmax over 9 channels of spg [9,512,960] -> disp1 [512,960]

Architecture:
  - top-2: DVE max8 + max_index per w column (exact fp32 ordering); the w
    axis is split in halves so the left half's fine stage overlaps the DVE
    top-2 stream of the right half.
  - fine stage quads: quad q covers fine rows 56q..56q+55; SBUF layout
    [126 part = (c, yl), 4 k, 960 x] with fine row = 56q + 4*yl + k, so
    y4 = 14q + yl exactly: all 4 k-slices share one upsampled patch tensor
    u_q [126, 960] built by TensorE from disp4p column windows
    (sliding-window-masked 0/1 lhsT, accumulated over the 3 cj shifts).
    Channel sums of num/den are 0/1-matmuls into [56, *] PSUM accumulators
    (per-k lhsT scatters rows to 4*yl+k). ScalarE: exps, psum->sbuf
    expand/copies, reciprocals. p = e*u split DVE/GpSimd.
  - spg arrives as one SWDGE cast-DMA (f32->bf16 in flight) per quad;
    cost in 4 column chunks sized for an early top-2 start; one packed
    const DMA; per-(quad, half) output DMAs on the scalar ring.
"""

import os
import sys
from contextlib import ExitStack

import numpy as np
import ml_dtypes

if "/opt/trn_rl_repo" not in sys.path:
    sys.path.insert(0, "/opt/trn_rl_repo")

import concourse.bass as bass
import concourse.bacc as bacc
import concourse.tile as tile
from concourse import mybir
from concourse.bass_utils import run_bass_kernel_spmd

F32 = mybir.dt.float32
BF16 = mybir.dt.bfloat16
U16 = mybir.dt.uint16
OP = mybir.AluOpType
ACT = mybir.ActivationFunctionType

B, D, H, W = 8, 48, 128, 240
HF, WF = 4 * H, 4 * W  # 512, 960
N_CORES = 8
NQ = 9            # quads of 4x14 interleaved rows; rows 504..511 = "q=9"
HALF = 480        # fine columns per half
CHALF = 120       # coarse columns per half

COST_CHUNKS = [(0, 32), (32, 122), (122, 182), (182, 240)]

# const packing (bf16 words per partition): wuz | wsumk | w36
WUZ_W = 10 * 154
WSK_W = 4 * 56
CONST_W = WUZ_W + WSK_W + 8


def mul_on_gp(q, k, half):
    return k >= 2 and (q + half) % 2 == 0


def div_on_gp(q, half):
    return False  # GPSIMD cannot read PSUM


def is_coarse(q):
    return q % 2 == 0


def build_consts():
    c = np.zeros((128, CONST_W), dtype=ml_dtypes.bfloat16)
    wuz = c[:, :WUZ_W].reshape(128, 10, 154)
    for q in range(NQ):
        for ci in range(3):
            for yl in range(14):
                r = 14 * q + yl + ci - 1
                if 0 <= r <= 127:
                    wuz[r, q, 28 + 42 * ci + yl] = 1.0
    for ci in range(3):  # rows 504..511 block at slot 9 (pad 16, 24-wide base)
        for yl in range(8):
            r = 126 + yl // 4 + ci - 1
            if 0 <= r <= 127:
                wuz[r, 9, 16 + 24 * ci + yl] = 1.0
    wsk = c[:126, WUZ_W:WUZ_W + WSK_W].reshape(126, 4, 56)
    for ch in range(9):
        for yl in range(14):
            for k in range(4):
                wsk[14 * ch + yl, k, 4 * yl + k] = 1.0
    w36 = c[:72, WUZ_W + WSK_W:]
    for ch in range(9):
        for yl in range(8):
            w36[8 * ch + yl, yl] = 1.0
    return c


def _act_reciprocal(nc, out_ap, in_ap):
    eng = nc.scalar
    return eng.add_instruction(
        mybir.InstActivation(
            name=nc.get_next_instruction_name(),
            func=ACT.Reciprocal,
            ins=[
                eng.lower_ap(in_ap),
                mybir.ImmediateValue(dtype=F32, value=0.0),
                mybir.ImmediateValue(dtype=F32, value=1.0),
                mybir.ImmediateValue(dtype=F32, value=0.0),
            ],
            outs=[eng.lower_ap(out_ap)],
        )
    )


def build_kernel(ctx, tc, out_d, cost_d, spg_d, const_d):
    nc = tc.nc
    cost_hdw = cost_d.transpose([1, 0, 2])  # [128(h), 48(d), 240(w)] view

    persist = ctx.enter_context(tc.tile_pool(name="persist", bufs=1))
    c_pool = ctx.enter_context(tc.tile_pool(name="cpool", bufs=2))
    raw_pool = ctx.enter_context(tc.tile_pool(name="raw", bufs=2))
    e_pool = ctx.enter_context(tc.tile_pool(name="epool", bufs=NQ + 1))
    den_pool = ctx.enter_context(tc.tile_pool(name="densb", bufs=NQ + 2))
    u_pool = ctx.enter_context(tc.tile_pool(name="usb", bufs=3))
    p_pool = ctx.enter_context(tc.tile_pool(name="ppool", bufs=4))
    r_pool = ctx.enter_context(tc.tile_pool(name="rpool", bufs=2))
    o_pool = ctx.enter_context(tc.tile_pool(name="opool", bufs=3))
    acc_ps = ctx.enter_context(tc.tile_pool(name="accps", bufs=2, space="PSUM"))
    u_ps_pool = ctx.enter_context(tc.tile_pool(name="ups", bufs=2, space="PSUM"))

    # ---- persistent tiles ----
    v8 = persist.tile([128, W, 8], F32)
    i8 = persist.tile([128, W, 8], U16)
    disp4p = persist.tile([128, W + 2], BF16)
    bl = [persist.tile([128, W], F32, name=f"bl{i}", tag=f"bl{i}")
          for i in range(6)]
    const_sb = persist.tile([128, CONST_W], BF16)
    wuz_v = const_sb[:, :WUZ_W].rearrange("p (q m) -> p q m", q=10)
    wsk_v = const_sb[:126, WUZ_W:WUZ_W + WSK_W].rearrange(
        "p (k m) -> p k m", k=4)
    w36_v = const_sb[:72, WUZ_W + WSK_W:]

    # ---- const DMA (scalar ring) + cost DMAs (sync ring) ----
    nc.scalar.dma_start(const_sb[:], const_d)
    nc.vector.memset(disp4p[:], 0.0)
    cost_tiles = []
    for (a, b) in COST_CHUNKS:
        ct = c_pool.tile([128, D, b - a], F32, tag="c")
        nc.sync.dma_start(ct[:], cost_hdw[:, :, a:b])
        cost_tiles.append(ct)

    # ---- spg quad DMAs: SWDGE cast f32->bf16 ----
    # raw_q [126, 5(pad), 964(pad)] bf16; partition (c, yl); fine row =
    # 56q + 4*yl + k.  src [9, 14, 3840] (c, yl, (k x)) stays 3-dim.
    raw_tiles = {}
    for q in range(NQ):
        raw = raw_pool.tile([126, 5, 964], BF16, tag="raw")
        src = spg_d[:, 56 * q:56 * q + 56, :].rearrange(
            "c (yl k) x -> c yl (k x)", k=4)
        nc.gpsimd.dma_start(raw[:, 0:4, 0:960], src)
        raw_tiles[q] = raw
    raw36 = persist.tile([72, 964], BF16)
    nc.gpsimd.dma_start(raw36[:, 0:960], spg_d[:, 504:512, :])

    # ---- exps (SE) + den matmuls (TE) + den copies (SE) ----
    e_tiles = {}
    den_sb = {}
    for q in range(NQ):
        e_q = e_pool.tile([126, 4, WF], BF16, name=f"e{q}", tag="e")
        nc.scalar.activation(e_q[:], raw_tiles[q][:, 0:4, 0:960], ACT.Exp)
        e_tiles[q] = e_q
    # (tile 36 is registered as e_tiles[9] below)
        dps = acc_ps.tile([56, WF], F32, name=f"denps{q}", tag="acc")
        for k in range(4):
            for (n0, nn) in ((0, 512), (512, 448)):
                nc.tensor.matmul(
                    dps[:, n0:n0 + nn], wsk_v[:, k, :], e_q[:, k, n0:n0 + nn],
                    start=(k == 0), stop=(k == 3),
                )
        dsb = den_pool.tile([56, WF], BF16, name=f"dsb{q}", tag="dsb")
        nc.scalar.copy(dsb[:], dps[:])
        den_sb[q] = dsb
    e36 = e_pool.tile([126, 4, WF], BF16, name="e36", tag="e")
    e_tiles[9] = e36
    nc.scalar.activation(e36[:72, 0, :], raw36[:, 0:960], ACT.Exp)
    dps36 = acc_ps.tile([8, WF], F32, name="denps36", tag="acc")
    for (n0, nn) in ((0, 512), (512, 448)):
        nc.tensor.matmul(dps36[:, n0:n0 + nn], w36_v, e36[:72, 0, n0:n0 + nn],
                         start=True, stop=True)
    dsb36 = den_pool.tile([8, WF], BF16, name="dsb36", tag="dsb")
    nc.scalar.copy(dsb36[:], dps36[:])
    den_sb[9] = dsb36

    # ---- top-2 stream (DVE) + per-half blend ----
    i1f, i2f, dl, tx, dn, rc = bl

    def head_chunk(ci_):
        a, b = COST_CHUNKS[ci_]
        ct = cost_tiles[ci_]
        for w in range(a, b):
            nc.vector.max(out=v8[:, w], in_=ct[:, :, w - a])
        for w in range(a, b):
            nc.vector.max_index(i8[:, w], v8[:, w], ct[:, :, w - a])

    def blend(a, b):
        s = slice(a, b)
        nc.vector.tensor_copy(i1f[:, s], i8[:, s, 0])
        nc.vector.tensor_copy(i2f[:, s], i8[:, s, 1])
        nc.vector.tensor_sub(dl[:, s], v8[:, s, 1], v8[:, s, 0])
        nc.scalar.activation(tx[:, s], dl[:, s], ACT.Exp)
        nc.vector.tensor_scalar_add(dn[:, s], tx[:, s], 1.0)
        nc.vector.reciprocal(rc[:, s], dn[:, s])
        nc.vector.tensor_mul(dl[:, s], tx[:, s], i2f[:, s])
        nc.vector.tensor_add(dl[:, s], dl[:, s], i1f[:, s])
        nc.vector.tensor_mul(disp4p[:, 1 + a:1 + b], dl[:, s], rc[:, s])

    # ---- fine-stage per (half, quad) ----
    def emit_u(q, half):
        M = 126 if q < 9 else 72
        shift = 14 if q < 9 else 8
        pad = 28 if q < 9 else 16
        c0 = CHALF * half
        if is_coarse(q):
            ups = u_ps_pool.tile([M, CHALF], F32, name=f"ups{q}", tag="u")
            for cj in range(3):
                s = pad - shift * cj
                nc.tensor.matmul(
                    ups[:], wuz_v[:, q, s:s + M],
                    disp4p[:, cj + c0:cj + c0 + CHALF],
                    start=(cj == 0), stop=(cj == 2),
                )
            usb = u_pool.tile([M, HALF], BF16, tag="u")
            nc.scalar.copy(
                usb[:].rearrange("m (a b) -> m a b", b=4),
                ups[:].unsqueeze(2).broadcast_to([M, CHALF, 4]),
            )
        else:
            ups = u_ps_pool.tile([M, HALF], F32, name=f"upsf{q}", tag="u")
            for cj in range(3):
                s = pad - shift * cj
                rhs = disp4p[:, cj + c0:cj + c0 + CHALF]
                rhs = rhs.unsqueeze(2).broadcast_to([128, CHALF, 4])
                nc.tensor.matmul(
                    ups[:], wuz_v[:, q, s:s + M], rhs,
                    start=(cj == 0), stop=(cj == 2),
                )
            usb = u_pool.tile([M, HALF], BF16, tag="u")
            nc.scalar.copy(usb[:], ups[:])
        return usb

    def emit_quad(q, half):
        cs = slice(HALF * half, HALF * (half + 1))
        usb = emit_u(q, half)
        if q < 9:
            nps = acc_ps.tile([56, HALF], F32, name=f"nps{q}_{half}",
                              tag="acc")
            for k in range(4):
                p_t = p_pool.tile([126, HALF], BF16, tag="p")
                eng = nc.gpsimd if mul_on_gp(q, k, half) else nc.vector
                eng.tensor_mul(p_t[:], e_tiles[q][:, k, cs], usb[:])
                nc.tensor.matmul(nps[:], wsk_v[:, k, :], p_t[:],
                                 start=(k == 0), stop=(k == 3))
            rows, r0 = 56, 56 * q
        else:
            nps = acc_ps.tile([8, HALF], F32, name=f"nps36_{half}", tag="acc")
            p_t = p_pool.tile([72, HALF], BF16, tag="p")
            eng = nc.gpsimd if mul_on_gp(q, 0, half) else nc.vector
            eng.tensor_mul(p_t[:], e_tiles[9][:72, 0, cs], usb[:])
            nc.tensor.matmul(nps[:], w36_v, p_t[:], start=True, stop=True)
            rows, r0 = 8, 504
        rden = r_pool.tile([rows, HALF], F32, tag="rd")
        _act_reciprocal(nc, rden[:], den_sb[q][:, cs])
        osb = o_pool.tile([rows, HALF], F32, tag="o")
        deng = nc.gpsimd if div_on_gp(q, half) else nc.vector
        deng.scalar_tensor_tensor(
            osb[:], nps[:], 4.0, rden[:], op0=OP.mult, op1=OP.mult,
        )
        nc.scalar.dma_start(out_d[r0:r0 + rows, cs], osb[:])

    head_chunk(0)
    head_chunk(1)
    blend(0, 122)
    for q in range(10):
        emit_quad(q, 0)
    head_chunk(2)
    head_chunk(3)
    blend(122, 240)
    for q in range(10):
        emit_quad(q, 1)


def build_program():
    nc = bacc.Bacc(
        "TRN2",
        target_bir_lowering=False,
        debug=False,
        enable_asserts=False,
        num_devices=N_CORES,
    )
    cost_d = nc.dram_tensor("cost", [D, H, W], F32, kind="ExternalInput").ap()
    spg_d = nc.dram_tensor("spg", [9, HF, WF], F32, kind="ExternalInput").ap()
    const_d = nc.dram_tensor("konst", [128, CONST_W], BF16,
                             kind="ExternalInput").ap()
    out_d = nc.dram_tensor("out", [HF, WF], F32, kind="ExternalOutput").ap()
    with tile.TileContext(nc) as tc:
        with ExitStack() as ctx:
            build_kernel(ctx, tc, out_d, cost_d, spg_d, const_d)
    nc.compile()
    return nc


def _install_ntff_hook():
    import types

    if "antenv.axon_hooks" in sys.modules:
        return True
    try:
        import antenv
        from trn_agent_boot.trn_boot import _ntff_profile_via_ctypes

        mod = types.ModuleType("antenv.axon_hooks")
        mod._hook = None

        def set_axon_ntff_profile_hook(hook):
            mod._hook = hook

        def get_axon_ntff_profile_hook():
            return mod._hook

        mod.set_axon_ntff_profile_hook = set_axon_ntff_profile_hook
        mod.get_axon_ntff_profile_hook = get_axon_ntff_profile_hook
        sys.modules["antenv.axon_hooks"] = mod
        antenv.axon_hooks = mod
        mod._hook = _ntff_profile_via_ctypes("/opt/axon/libaxon_pjrt.so")
        return True
    except Exception as e:  # profiling is best-effort
        print(f"NTFF hook install failed: {e}")
        return False


LAST_RESULTS = None


def kernel(cost: np.ndarray, spg: np.ndarray) -> np.ndarray:
    """cost [8,1,48,128,240] f32, spg [8,9,512,960] f32 -> disp1 [8,512,960]."""
    global LAST_RESULTS
    cost = np.ascontiguousarray(np.asarray(cost, dtype=np.float32))
    spg = np.ascontiguousarray(np.asarray(spg, dtype=np.float32))
    assert cost.shape == (B, 1, D, H, W) and spg.shape == (B, 9, HF, WF)

    konst = build_consts()
    nc = build_program()
    in_maps = [
        {"cost": cost[b, 0], "spg": spg[b], "konst": konst}
        for b in range(B)
    ]
    trace = bool(int(os.environ.get("KERNEL_TRACE", "0")))
    if trace:
        trace = _install_ntff_hook()
    res = run_bass_kernel_spmd(
        nc, in_maps, core_ids=list(range(N_CORES)), trace=trace
    )
    LAST_RESULTS = res
    out = np.stack([res.results[b]["out"] for b in range(B)], axis=0)
    return out.astype(np.float32, copy=False)
